# revision 10
# baseline (speedup 1.0000x reference)
"""Trainium2 Bass kernel for nn_CompositionalMlp (4-node compositional MLP,
4 experts/node, exact one-hot routing), data-parallel over batch on 8 cores.

Strategy: host-side global sort of rows by the (e1,e2,e3,e0) expert combo.
Each of the 256 combos is padded to a multiple of 8 and dealt round-robin to
the 8 cores, so every core sees the *same* column layout (one static SPMD
program) and each node's expert segments are contiguous column runs. Only
the routed expert is computed per row (4x fewer matmul passes than dense),
with no masks, no combines, and no on-device data movement between nodes.
Matmul cost on TRN2 is N_cols * cycle regardless of K/M, so the per-run
instruction fragmentation (node0 has ~256 tiny runs) is nearly free.
"""
import os
import sys
sys.path.insert(0, "/opt/trn_rl_repo")
os.environ.setdefault("NEURON_RT_RESET_CORES", "1")
import numpy as np

B = 65536
E = 4
F = 32
H = 256
NODES = 4
D_MID = 128
D_OUT = 8
N_CORES = 8
CH = 512

_COMPILED = {}
_PLAN_CACHE = {}


def _plan(input_val):
    key = id(input_val)
    oh = np.asarray(input_val)[:, NODES * F:].reshape(B, NODES, E)
    e = np.argmax(oh, axis=2).astype(np.int32)          # [B, 4]
    e0, e1, e2, e3 = e[:, 0], e[:, 1], e[:, 2], e[:, 3]
    ci = ((e1 * 4 + e2) * 4 + e3) * 4 + e0              # combo id, (e1,e2,e3,e0) lex
    cnt = np.bincount(ci, minlength=256)
    npc = (cnt + N_CORES - 1) // N_CORES                # per-core cols per combo
    off = np.zeros(257, np.int64)
    off[1:] = np.cumsum(npc)
    W = int(off[256])

    order = np.argsort(ci, kind="stable")               # rows grouped by combo
    col_of_row = np.empty(B, np.int64)
    core_of_row = np.empty(B, np.int64)
    pos = 0
    for c in range(256):
        n = int(cnt[c])
        rows = order[pos:pos + n]
        pos += n
        k = np.arange(n)
        core_of_row[rows] = k % N_CORES
        col_of_row[rows] = off[c] + k // N_CORES

    def ej(c, j):
        return (c & 3) if j == 0 else (c >> 6) if j == 1 \
            else ((c >> 4) & 3) if j == 2 else ((c >> 2) & 3)

    runs = []
    for j in range(4):
        rj = []
        for c in range(256):
            a, b = int(off[c]), int(off[c + 1])
            if a == b:
                continue
            x = ej(c, j)
            if rj and rj[-1][0] == x and rj[-1][2] == a:
                rj[-1] = (x, rj[-1][1], b)
            else:
                rj.append((x, a, b))
        runs.append(tuple(tuple(r) for r in rj))

    e3_of_col = np.zeros(W, np.int32)
    for c in range(256):
        e3_of_col[off[c]:off[c + 1]] = (c >> 2) & 3

    return dict(W=W, runs=tuple(runs), col_of_row=col_of_row,
                core_of_row=core_of_row, e3_of_col=e3_of_col)


def _pieces(runs_j, a0, b0):
    out = []
    for (x, a, b) in runs_j:
        if b <= a0 or a >= b0:
            continue
        out.append((x, max(a, a0), min(b, b0)))
    return out


def _build(W, runs, bpre, bmid, bup):
    import concourse.bass as bass  # noqa: F401
    from concourse import bacc
    import concourse.mybir as mybir
    from concourse.tile import TileContext

    F32 = mybir.dt.float32
    F16 = mybir.dt.float16
    ADD = mybir.AluOpType.add
    MAX = mybir.AluOpType.max

    nc = bacc.Bacc("TRN2", target_bir_lowering=False, debug=False,
                   num_devices=N_CORES)
    xfall = nc.dram_tensor("xfall", [128, W], F16, kind="ExternalInput").ap()
    win = nc.dram_tensor("win", [16, 32, 256], F16, kind="ExternalInput").ap()
    wa = nc.dram_tensor("wa", [12, 128, 256], F16, kind="ExternalInput").ap()
    wb = nc.dram_tensor("wb", [12, 256, 256], F16, kind="ExternalInput").ap()
    wu = nc.dram_tensor("wu", [12, 256, 128], F16, kind="ExternalInput").ap()
    w3u = nc.dram_tensor("w3u", [4, 256, 8], F16, kind="ExternalInput").ap()
    bvals = nc.dram_tensor("bvals", [128, 10], F32, kind="ExternalInput").ap()
    yT = nc.dram_tensor("yT", [8, W], F32, kind="ExternalOutput").ap()

    chunks = [(a, min(a + CH, W)) for a in range(0, W, CH)]
    nch = len(chunks)

    with TileContext(nc) as tc:
        with (
            tc.tile_pool(name="wpool", bufs=1) as wp,
            tc.tile_pool(name="big", bufs=1) as bigp,
            tc.tile_pool(name="xfp", bufs=4) as xfp,
            tc.tile_pool(name="acts", bufs=1) as ap_,
            tc.tile_pool(name="ppre", bufs=2, space="PSUM") as ppre,
            tc.tile_pool(name="pmid", bufs=1, space="PSUM") as pmid,
            tc.tile_pool(name="pup", bufs=1, space="PSUM") as pup,
            tc.tile_pool(name="phd", bufs=1, space="PSUM") as phd,
        ):
            def wtile(shape, tag, src):
                t = wp.tile(shape, F16, tag=tag, name=tag)
                nc.sync.dma_start(t[:, :], src)
                return t

            win_t = [wtile([32, 256], f"win{i}", win[i]) for i in range(16)]
            wa_t = [wtile([128, 256], f"wa{i}", wa[i]) for i in range(12)]
            wb_t = [[wtile([128, 256], f"wb{i}_{k}", wb[i][k * 128:(k + 1) * 128, :])
                     for k in range(2)] for i in range(12)]
            wu_t = [[wtile([128, 128], f"wu{i}_{k}", wu[i][k * 128:(k + 1) * 128, :])
                     for k in range(2)] for i in range(12)]
            w3u_t = [[wtile([128, 8], f"w3u{e}_{k}", w3u[e][k * 128:(k + 1) * 128, :])
                      for k in range(2)] for e in range(4)]

            xp_t = [bigp.tile([128, W], F16, tag=f"xp{j}", name=f"xp{j}")
                    for j in range(3)]
            bv_t = bigp.tile([128, 10], F32, tag="bvals", name="bv_t")
            nc.sync.dma_start(bv_t[:, :], bvals[:, :])
            BIDX = {}
            for _i, _k in enumerate(("p0", "p1", "p2", "p3", "m1", "m2", "m3",
                                     "u0", "u1", "u2")):
                BIDX[_k] = _i

            state = {}
            HB = {0: 3, 1: 4, 2: 6, 3: 8}      # hin buf depth per node
            RELU = mybir.ActivationFunctionType.Relu
            eng_busy = {"vector": 0.0, "scalar": 0.0}
            EW = {"vector": (1.04, 170.0), "scalar": (0.833, 175.0)}

            def emit_relu(out_ap, in_ap, bias, nfree, bkey):
                costs = {e: nfree * w + f for e, (w, f) in EW.items()}
                e = min(eng_busy, key=lambda k: eng_busy[k] + costs[k])
                eng_busy[e] += costs[e]
                if e == "scalar":
                    bi = BIDX[bkey]
                    nc.scalar.activation(out_ap, in_ap, RELU,
                                         bias=bv_t[0:in_ap.partition_size(),
                                                   bi:bi + 1])
                else:
                    getattr(nc, e).tensor_scalar(out_ap, in_ap, float(bias),
                                                 0.0, ADD, MAX)

            def fetch_xf(j, ci):
                if ci >= nch or (j, ci) in state:
                    return
                a0, b0 = chunks[ci]
                t = xfp.tile([32, CH], F16, tag=f"xf{j}", name=f"xf{j}")
                nc.sync.dma_start(t[:, 0:b0 - a0], xfall[32 * j:32 * (j + 1), a0:b0])
                state[(j, ci)] = t

            def emit_pre(j, ci):
                a0, b0 = chunks[ci]
                L = b0 - a0
                if ci == 0:
                    fetch_xf(j, 0)
                    fetch_xf(j, 1)
                fetch_xf(j, ci + 2)
                xt = state.pop((j, ci))
                ps = ppre.tile([128, 2, CH], F32, tag="pre", name="pre")
                for (ex, pa, pb) in _pieces(runs[j], a0, b0):
                    ra, rb = pa - a0, pb - a0
                    for m in (0, 1):
                        nc.tensor.matmul(
                            ps[:, m, ra:rb],
                            win_t[4 * j + ex][:, m * 128:(m + 1) * 128],
                            xt[:, ra:rb], start=True, stop=True)
                h = ap_.tile([128, 2, CH], F16, tag=f"h{j}", name=f"h{j}",
                             bufs=HB[j])
                emit_relu(h[:, :, 0:L], ps[:, :, 0:L], bpre[j], 2 * L, f"p{j}")
                state[("h", j, ci)] = h

            def emit_mid(j, ci):
                a0, b0 = chunks[ci]
                L = b0 - a0
                hp = state.pop(("h", j, ci))
                ps = pmid.tile([128, 2, CH], F32, tag="mid", name="mid")
                for (ex, pa, pb) in _pieces(runs[j], a0, b0):
                    ra, rb = pa - a0, pb - a0
                    i12 = 4 * (j - 1) + ex
                    for m in (0, 1):
                        sl = slice(m * 128, (m + 1) * 128)
                        nc.tensor.matmul(ps[:, m, ra:rb], wb_t[i12][0][:, sl],
                                         hp[:, 0, ra:rb], start=True, stop=False)
                        nc.tensor.matmul(ps[:, m, ra:rb], wb_t[i12][1][:, sl],
                                         hp[:, 1, ra:rb], start=False, stop=False)
                        nc.tensor.matmul(ps[:, m, ra:rb], wa_t[i12][:, sl],
                                         xp_t[j - 1][:, pa:pb],
                                         start=False, stop=True)
                h = ap_.tile([128, 2, CH], F16, tag=f"hm{j}", name=f"hm{j}",
                             bufs=3)
                emit_relu(h[:, :, 0:L], ps[:, :, 0:L], bmid[j], 2 * L, f"m{j}")
                state[("hm", j, ci)] = h

            def emit_up(j, ci):
                a0, b0 = chunks[ci]
                L = b0 - a0
                hp = state.pop(("hm", j, ci) if j > 0 else ("h", j, ci))
                ps = pup.tile([128, CH], F32, tag="up", name="up")
                for (ex, pa, pb) in _pieces(runs[j], a0, b0):
                    ra, rb = pa - a0, pb - a0
                    nc.tensor.matmul(ps[:, ra:rb], wu_t[4 * j + ex][0][:, :],
                                     hp[:, 0, ra:rb], start=True, stop=False)
                    nc.tensor.matmul(ps[:, ra:rb], wu_t[4 * j + ex][1][:, :],
                                     hp[:, 1, ra:rb], start=False, stop=True)
                emit_relu(xp_t[j][:, a0:b0], ps[:, 0:L], bup[j], L, f"u{j}")

            def emit_head(ci):
                a0, b0 = chunks[ci]
                L = b0 - a0
                hp = state.pop(("hm", 3, ci))
                ps = phd.tile([8, CH], F32, tag="hd", name="hd")
                for (ex, pa, pb) in _pieces(runs[3], a0, b0):
                    ra, rb = pa - a0, pb - a0
                    nc.tensor.matmul(ps[:, ra:rb], w3u_t[ex][0][:, :],
                                     hp[:, 0, ra:rb], start=True, stop=False)
                    nc.tensor.matmul(ps[:, ra:rb], w3u_t[ex][1][:, :],
                                     hp[:, 1, ra:rb], start=False, stop=True)
                yc = ap_.tile([8, CH], F32, tag="yc", name="yc", bufs=2)
                nc.any.tensor_scalar(yc[:, 0:L], ps[:, 0:L], 0.0, None, ADD)
                nc.sync.dma_start(yT[:, a0:b0], yc[:, 0:L])

            STAGES = [
                (lambda c: emit_pre(0, c), 0),
                (lambda c: emit_up(0, c), 1),
                (lambda c: emit_pre(1, c), 0),
                (lambda c: emit_mid(1, c), 2),
                (lambda c: emit_pre(2, c), 0),
                (lambda c: emit_up(1, c), 3),
                (lambda c: emit_pre(3, c), 0),
                (lambda c: emit_mid(2, c), 4),
                (lambda c: emit_up(2, c), 5),
                (lambda c: emit_mid(3, c), 6),
                (lambda c: emit_head(c), 7),
            ]
            for step in range(nch + 8):
                for fn, skew in STAGES:
                    c = step - skew
                    if 0 <= c < nch:
                        fn(c)
    nc.compile()
    return nc


def _bias_scalar(b):
    b = np.asarray(b, np.float32)
    assert np.ptp(b) == 0.0, "bias must be a single constant"
    return float(b.flat[0])


def _prep_inputs(p):
    f16 = np.float16
    plan = _plan(p["input_val"])
    W = plan["W"]

    win = np.empty((16, 32, 256), f16)
    in_w = [p["W0_0"], p["W1_pre"], p["W2_pre"], p["W3_pre"]]
    for j in range(4):
        for e in range(4):
            win[4 * j + e] = in_w[j][e]

    wa = np.empty((12, 128, 256), f16)
    wb = np.empty((12, 256, 256), f16)
    for j in (1, 2, 3):
        w0 = p[f"W{j}_0"]
        for e in range(4):
            i12 = 4 * (j - 1) + e
            wa[i12] = w0[e][0:128, :]
            wb[i12] = w0[e][128:384, :]

    wu = np.empty((12, 256, 128), f16)
    for j in (0, 1, 2):
        w1 = p[f"W{j}_1"]
        for e in range(4):
            wu[4 * j + e] = w1[e]

    w3u = np.asarray(p["W3_1"]).astype(f16)

    xs = np.asarray(p["input_val"])[:, 0:NODES * F].astype(f16)     # [B, 128]

    bv = np.zeros((128, 10), np.float32)
    for i, k in enumerate(("b0_0", "b1_pre", "b2_pre", "b3_pre",
                           "b1_0", "b2_0", "b3_0",
                           "b0_1", "b1_1", "b2_1")):
        bv[:, i] = np.asarray(p[k], np.float32).flat[0]
    shared = dict(win=win, wa=wa, wb=wb, wu=wu, w3u=w3u, bvals=bv)
    in_maps = []
    for c in range(N_CORES):
        rows = np.where(plan["core_of_row"] == c)[0]
        cols = plan["col_of_row"][rows]
        xf = np.zeros((128, W), f16)
        xf[:, cols] = xs[rows].T
        m = dict(shared)
        m["xfall"] = xf
        in_maps.append(m)
    return in_maps


def kernel(**inputs):
    from concourse.bass_utils import run_bass_kernel_spmd

    p = {k: np.asarray(v) for k, v in inputs.items()}
    plan = _plan(p["input_val"])
    bpre = [_bias_scalar(p[k]) for k in ("b0_0", "b1_pre", "b2_pre", "b3_pre")]
    bmid = {j: _bias_scalar(p[f"b{j}_0"]) for j in (1, 2, 3)}
    bup = {j: _bias_scalar(p[f"b{j}_1"]) for j in (0, 1, 2)}

    key = (plan["W"], plan["runs"], tuple(bpre),
           tuple(sorted(bmid.items())), tuple(sorted(bup.items())))
    if key not in _COMPILED:
        _COMPILED[key] = _build(plan["W"], plan["runs"], bpre, bmid, bup)
        _COMPILED[("nc", True)] = _COMPILED[key]
    nc = _COMPILED[key]

    in_maps = _prep_inputs(p)
    res = run_bass_kernel_spmd(nc, in_maps, core_ids=list(range(N_CORES)))

    out = np.empty((B, D_OUT), np.float32)
    for c in range(N_CORES):
        y = res.results[c]["yT"]                                    # [8, W]
        rows = np.where(plan["core_of_row"] == c)[0]
        cols = plan["col_of_row"][rows]
        out[rows] = y[:, cols].T
    oh = p["input_val"][:, NODES * F:].reshape(B, NODES, E)
    e3 = np.argmax(oh[:, 3], axis=1)
    out += np.asarray(p["b3_1"], np.float32)[e3]
    return out


# revision 11
# speedup vs baseline: 1.2515x; 1.2515x over previous
"""Trainium2 Bass kernel for nn_CompositionalMlp (4-node compositional MLP,
4 experts/node, exact one-hot routing), data-parallel over batch on 8 cores.

Strategy: host-side global sort of rows by the (e1,e2,e3,e0) expert combo.
Each of the 256 combos is padded to a multiple of 8 and dealt round-robin to
the 8 cores, so every core sees the *same* column layout (one static SPMD
program) and each node's expert segments are contiguous column runs. Only
the routed expert is computed per row (4x fewer matmul passes than dense),
with no masks, no combines, and no on-device data movement between nodes.
Matmul cost on TRN2 is N_cols * cycle regardless of K/M, so the per-run
instruction fragmentation (node0 has ~256 tiny runs) is nearly free.
"""
import os
import sys
sys.path.insert(0, "/opt/trn_rl_repo")
os.environ.setdefault("NEURON_RT_RESET_CORES", "1")
import numpy as np

B = 65536
E = 4
F = 32
H = 256
NODES = 4
D_MID = 128
D_OUT = 8
N_CORES = 8
CH = 512

_COMPILED = {}
_PLAN_CACHE = {}


def _plan(input_val):
    key = id(input_val)
    oh = np.asarray(input_val)[:, NODES * F:].reshape(B, NODES, E)
    e = np.argmax(oh, axis=2).astype(np.int32)          # [B, 4]
    e0, e1, e2, e3 = e[:, 0], e[:, 1], e[:, 2], e[:, 3]
    ci = ((e1 * 4 + e2) * 4 + e3) * 4 + e0              # combo id, (e1,e2,e3,e0) lex
    cnt = np.bincount(ci, minlength=256)
    npc = (cnt + N_CORES - 1) // N_CORES                # per-core cols per combo
    off = np.zeros(257, np.int64)
    off[1:] = np.cumsum(npc)
    W = int(off[256])

    order = np.argsort(ci, kind="stable")               # rows grouped by combo
    col_of_row = np.empty(B, np.int64)
    core_of_row = np.empty(B, np.int64)
    pos = 0
    for c in range(256):
        n = int(cnt[c])
        rows = order[pos:pos + n]
        pos += n
        k = np.arange(n)
        core_of_row[rows] = k % N_CORES
        col_of_row[rows] = off[c] + k // N_CORES

    def ej(c, j):
        return (c & 3) if j == 0 else (c >> 6) if j == 1 \
            else ((c >> 4) & 3) if j == 2 else ((c >> 2) & 3)

    runs = []
    for j in range(4):
        rj = []
        for c in range(256):
            a, b = int(off[c]), int(off[c + 1])
            if a == b:
                continue
            x = ej(c, j)
            if rj and rj[-1][0] == x and rj[-1][2] == a:
                rj[-1] = (x, rj[-1][1], b)
            else:
                rj.append((x, a, b))
        runs.append(tuple(tuple(r) for r in rj))

    e3_of_col = np.zeros(W, np.int32)
    for c in range(256):
        e3_of_col[off[c]:off[c + 1]] = (c >> 2) & 3

    return dict(W=W, runs=tuple(runs), col_of_row=col_of_row,
                core_of_row=core_of_row, e3_of_col=e3_of_col)


def _pieces(runs_j, a0, b0):
    out = []
    for (x, a, b) in runs_j:
        if b <= a0 or a >= b0:
            continue
        out.append((x, max(a, a0), min(b, b0)))
    return out


def _build(W, runs, bpre, bmid, bup):
    import concourse.bass as bass  # noqa: F401
    from concourse import bacc
    import concourse.mybir as mybir
    from concourse.tile import TileContext

    F32 = mybir.dt.float32
    F16 = mybir.dt.float16
    ADD = mybir.AluOpType.add
    MAX = mybir.AluOpType.max

    nc = bacc.Bacc("TRN2", target_bir_lowering=False, debug=False,
                   num_devices=N_CORES)
    xfall = nc.dram_tensor("xfall", [128, W], F16, kind="ExternalInput").ap()
    winall = nc.dram_tensor("winall", [32, 4096], F16, kind="ExternalInput").ap()
    waall = nc.dram_tensor("waall", [128, 3072], F16, kind="ExternalInput").ap()
    wball = nc.dram_tensor("wball", [128, 6144], F16, kind="ExternalInput").ap()
    wuall = nc.dram_tensor("wuall", [128, 3072], F16, kind="ExternalInput").ap()
    w3all = nc.dram_tensor("w3all", [128, 64], F16, kind="ExternalInput").ap()
    bvals = nc.dram_tensor("bvals", [128, 10], F32, kind="ExternalInput").ap()
    yT = nc.dram_tensor("yT", [8, W], F32, kind="ExternalOutput").ap()

    chunks = [(a, min(a + CH, W)) for a in range(0, W, CH)]
    nch = len(chunks)

    with TileContext(nc) as tc:
        with (
            tc.tile_pool(name="wpool", bufs=1) as wp,
            tc.tile_pool(name="big", bufs=1) as bigp,
            tc.tile_pool(name="xfp", bufs=4) as xfp,
            tc.tile_pool(name="acts", bufs=1) as ap_,
            tc.tile_pool(name="ppre", bufs=2, space="PSUM") as ppre,
            tc.tile_pool(name="pmid", bufs=1, space="PSUM") as pmid,
            tc.tile_pool(name="pup", bufs=1, space="PSUM") as pup,
            tc.tile_pool(name="phd", bufs=1, space="PSUM") as phd,
        ):
            def wtile(shape, tag, src):
                t = wp.tile(shape, F16, tag=tag, name=tag)
                nc.sync.dma_start(t[:, :], src)
                return t

            winall_t = wtile([32, 4096], "winall", winall[:, :])

            state = {}

            def fetch_xf(j, ci):
                if ci >= nch or (j, ci) in state:
                    return
                a0, b0 = chunks[ci]
                t = xfp.tile([32, CH], F16, tag=f"xf{j}", name=f"xf{j}")
                nc.sync.dma_start(t[:, 0:b0 - a0], xfall[32 * j:32 * (j + 1), a0:b0])
                state[(j, ci)] = t

            for _c in (0, 1):
                for _j in range(4):
                    fetch_xf(_j, _c)
            wu_all = wtile([128, 3072], "wuall", wuall[:, :])
            wb_all = wtile([128, 6144], "wball", wball[:, :])
            wa_all = wtile([128, 3072], "waall", waall[:, :])
            w3_all = wtile([128, 64], "w3all", w3all[:, :])

            def win_t(i):
                return winall_t[:, i * 256:(i + 1) * 256]

            def wb_t(i, k):
                return wb_all[:, (2 * i + k) * 256:(2 * i + k + 1) * 256]

            def wa_t(i):
                return wa_all[:, i * 256:(i + 1) * 256]

            def wu_t(i, k):
                return wu_all[:, (2 * i + k) * 128:(2 * i + k + 1) * 128]

            def w3u_t(e, k):
                return w3_all[:, (2 * e + k) * 8:(2 * e + k + 1) * 8]

            xp_t = [bigp.tile([128, W], F16, tag=f"xp{j}", name=f"xp{j}")
                    for j in range(3)]
            bv_t = bigp.tile([128, 10], F32, tag="bvals", name="bv_t")
            nc.sync.dma_start(bv_t[:, :], bvals[:, :])
            BIDX = {}
            for _i, _k in enumerate(("p0", "p1", "p2", "p3", "m1", "m2", "m3",
                                     "u0", "u1", "u2")):
                BIDX[_k] = _i

            HB = {0: 3, 1: 4, 2: 6, 3: 8}      # hin buf depth per node
            RELU = mybir.ActivationFunctionType.Relu
            eng_busy = {"vector": 0.0, "scalar": 0.0}
            EW = {"vector": (1.04, 170.0), "scalar": (0.833, 175.0)}

            def emit_relu(out_ap, in_ap, bias, nfree, bkey):
                costs = {e: nfree * w + f for e, (w, f) in EW.items()}
                e = min(eng_busy, key=lambda k: eng_busy[k] + costs[k])
                eng_busy[e] += costs[e]
                if e == "scalar":
                    bi = BIDX[bkey]
                    nc.scalar.activation(out_ap, in_ap, RELU,
                                         bias=bv_t[0:in_ap.partition_size(),
                                                   bi:bi + 1])
                else:
                    getattr(nc, e).tensor_scalar(out_ap, in_ap, float(bias),
                                                 0.0, ADD, MAX)

            def emit_pre(j, ci):
                a0, b0 = chunks[ci]
                L = b0 - a0
                if ci == 0:
                    fetch_xf(j, 0)
                    fetch_xf(j, 1)
                fetch_xf(j, ci + 2)
                xt = state.pop((j, ci))
                ps = ppre.tile([128, 2, CH], F32, tag="pre", name="pre")
                for (ex, pa, pb) in _pieces(runs[j], a0, b0):
                    ra, rb = pa - a0, pb - a0
                    for m in (0, 1):
                        nc.tensor.matmul(
                            ps[:, m, ra:rb],
                            win_t(4 * j + ex)[:, m * 128:(m + 1) * 128],
                            xt[:, ra:rb], start=True, stop=True)
                h = ap_.tile([128, 2, CH], F16, tag=f"h{j}", name=f"h{j}",
                             bufs=HB[j])
                emit_relu(h[:, :, 0:L], ps[:, :, 0:L], bpre[j], 2 * L, f"p{j}")
                state[("h", j, ci)] = h

            def emit_mid(j, ci):
                a0, b0 = chunks[ci]
                L = b0 - a0
                hp = state.pop(("h", j, ci))
                ps = pmid.tile([128, 2, CH], F32, tag="mid", name="mid")
                for (ex, pa, pb) in _pieces(runs[j], a0, b0):
                    ra, rb = pa - a0, pb - a0
                    i12 = 4 * (j - 1) + ex
                    for m in (0, 1):
                        sl = slice(m * 128, (m + 1) * 128)
                        nc.tensor.matmul(ps[:, m, ra:rb], wb_t(i12, 0)[:, sl],
                                         hp[:, 0, ra:rb], start=True, stop=False)
                        nc.tensor.matmul(ps[:, m, ra:rb], wb_t(i12, 1)[:, sl],
                                         hp[:, 1, ra:rb], start=False, stop=False)
                        nc.tensor.matmul(ps[:, m, ra:rb], wa_t(i12)[:, sl],
                                         xp_t[j - 1][:, pa:pb],
                                         start=False, stop=True)
                h = ap_.tile([128, 2, CH], F16, tag=f"hm{j}", name=f"hm{j}",
                             bufs=3)
                emit_relu(h[:, :, 0:L], ps[:, :, 0:L], bmid[j], 2 * L, f"m{j}")
                state[("hm", j, ci)] = h

            def emit_up(j, ci):
                a0, b0 = chunks[ci]
                L = b0 - a0
                hp = state.pop(("hm", j, ci) if j > 0 else ("h", j, ci))
                ps = pup.tile([128, CH], F32, tag="up", name="up")
                for (ex, pa, pb) in _pieces(runs[j], a0, b0):
                    ra, rb = pa - a0, pb - a0
                    nc.tensor.matmul(ps[:, ra:rb], wu_t(4 * j + ex, 0),
                                     hp[:, 0, ra:rb], start=True, stop=False)
                    nc.tensor.matmul(ps[:, ra:rb], wu_t(4 * j + ex, 1),
                                     hp[:, 1, ra:rb], start=False, stop=True)
                emit_relu(xp_t[j][:, a0:b0], ps[:, 0:L], bup[j], L, f"u{j}")

            def emit_head(ci):
                a0, b0 = chunks[ci]
                L = b0 - a0
                hp = state.pop(("hm", 3, ci))
                ps = phd.tile([8, CH], F32, tag="hd", name="hd")
                for (ex, pa, pb) in _pieces(runs[3], a0, b0):
                    ra, rb = pa - a0, pb - a0
                    nc.tensor.matmul(ps[:, ra:rb], w3u_t(ex, 0),
                                     hp[:, 0, ra:rb], start=True, stop=False)
                    nc.tensor.matmul(ps[:, ra:rb], w3u_t(ex, 1),
                                     hp[:, 1, ra:rb], start=False, stop=True)
                yc = ap_.tile([8, CH], F32, tag="yc", name="yc", bufs=2)
                nc.any.tensor_scalar(yc[:, 0:L], ps[:, 0:L], 0.0, None, ADD)
                nc.sync.dma_start(yT[:, a0:b0], yc[:, 0:L])

            STAGES = [
                (lambda c: emit_pre(0, c), 0),
                (lambda c: emit_up(0, c), 1),
                (lambda c: emit_pre(1, c), 0),
                (lambda c: emit_mid(1, c), 2),
                (lambda c: emit_pre(2, c), 0),
                (lambda c: emit_up(1, c), 3),
                (lambda c: emit_pre(3, c), 0),
                (lambda c: emit_mid(2, c), 4),
                (lambda c: emit_up(2, c), 5),
                (lambda c: emit_mid(3, c), 6),
                (lambda c: emit_head(c), 7),
            ]
            for step in range(nch + 8):
                for fn, skew in STAGES:
                    c = step - skew
                    if 0 <= c < nch:
                        fn(c)
    nc.compile()
    return nc


def _bias_scalar(b):
    b = np.asarray(b, np.float32)
    assert np.ptp(b) == 0.0, "bias must be a single constant"
    return float(b.flat[0])


def _prep_inputs(p):
    f16 = np.float16
    plan = _plan(p["input_val"])
    W = plan["W"]

    winall = np.empty((32, 4096), f16)
    in_w = [p["W0_0"], p["W1_pre"], p["W2_pre"], p["W3_pre"]]
    for j in range(4):
        for e in range(4):
            i = 4 * j + e
            winall[:, i * 256:(i + 1) * 256] = in_w[j][e]

    waall = np.empty((128, 3072), f16)
    wball = np.empty((128, 6144), f16)
    for j in (1, 2, 3):
        w0 = p[f"W{j}_0"]
        for e in range(4):
            i = 4 * (j - 1) + e
            waall[:, i * 256:(i + 1) * 256] = w0[e][0:128, :]
            for k in range(2):
                wball[:, (2 * i + k) * 256:(2 * i + k + 1) * 256] = \
                    w0[e][128 + 128 * k:256 + 128 * k, :]

    wuall = np.empty((128, 3072), f16)
    for j in (0, 1, 2):
        w1 = p[f"W{j}_1"]
        for e in range(4):
            i = 4 * j + e
            for k in range(2):
                wuall[:, (2 * i + k) * 128:(2 * i + k + 1) * 128] = \
                    w1[e][128 * k:128 * (k + 1), :]

    w3all = np.zeros((128, 64), f16)
    w3 = np.asarray(p["W3_1"]).astype(f16)
    for e in range(4):
        for k in range(2):
            w3all[:, (2 * e + k) * 8:(2 * e + k + 1) * 8] = \
                w3[e][128 * k:128 * (k + 1), :]

    xs = np.asarray(p["input_val"])[:, 0:NODES * F].astype(f16)     # [B, 128]

    bv = np.zeros((128, 10), np.float32)
    for i, k in enumerate(("b0_0", "b1_pre", "b2_pre", "b3_pre",
                           "b1_0", "b2_0", "b3_0",
                           "b0_1", "b1_1", "b2_1")):
        bv[:, i] = np.asarray(p[k], np.float32).flat[0]
    shared = dict(winall=winall, waall=waall, wball=wball, wuall=wuall,
                  w3all=w3all, bvals=bv)
    in_maps = []
    for c in range(N_CORES):
        rows = np.where(plan["core_of_row"] == c)[0]
        cols = plan["col_of_row"][rows]
        xf = np.zeros((128, W), f16)
        xf[:, cols] = xs[rows].T
        m = dict(shared)
        m["xfall"] = xf
        in_maps.append(m)
    return in_maps


def kernel(**inputs):
    from concourse.bass_utils import run_bass_kernel_spmd

    p = {k: np.asarray(v) for k, v in inputs.items()}
    plan = _plan(p["input_val"])
    bpre = [_bias_scalar(p[k]) for k in ("b0_0", "b1_pre", "b2_pre", "b3_pre")]
    bmid = {j: _bias_scalar(p[f"b{j}_0"]) for j in (1, 2, 3)}
    bup = {j: _bias_scalar(p[f"b{j}_1"]) for j in (0, 1, 2)}

    key = (plan["W"], plan["runs"], tuple(bpre),
           tuple(sorted(bmid.items())), tuple(sorted(bup.items())))
    if key not in _COMPILED:
        _COMPILED[key] = _build(plan["W"], plan["runs"], bpre, bmid, bup)
        _COMPILED[("nc", True)] = _COMPILED[key]
    nc = _COMPILED[key]

    in_maps = _prep_inputs(p)
    res = run_bass_kernel_spmd(nc, in_maps, core_ids=list(range(N_CORES)))

    out = np.empty((B, D_OUT), np.float32)
    for c in range(N_CORES):
        y = res.results[c]["yT"]                                    # [8, W]
        rows = np.where(plan["core_of_row"] == c)[0]
        cols = plan["col_of_row"][rows]
        out[rows] = y[:, cols].T
    oh = p["input_val"][:, NODES * F:].reshape(B, NODES, E)
    e3 = np.argmax(oh[:, 3], axis=1)
    out += np.asarray(p["b3_1"], np.float32)[e3]
    return out


# revision 12
# speedup vs baseline: 1.2818x; 1.0242x over previous
"""Trainium2 Bass kernel for nn_CompositionalMlp (4-node compositional MLP,
4 experts/node, exact one-hot routing), data-parallel over batch on 8 cores.

Strategy: host-side global sort of rows by the (e1,e2,e3,e0) expert combo.
Each of the 256 combos is padded to a multiple of 8 and dealt round-robin to
the 8 cores, so every core sees the *same* column layout (one static SPMD
program) and each node's expert segments are contiguous column runs. Only
the routed expert is computed per row (4x fewer matmul passes than dense),
with no masks, no combines, and no on-device data movement between nodes.
Matmul cost on TRN2 is N_cols * cycle regardless of K/M, so the per-run
instruction fragmentation (node0 has ~256 tiny runs) is nearly free.
"""
import os
import sys
sys.path.insert(0, "/opt/trn_rl_repo")
os.environ.setdefault("NEURON_RT_RESET_CORES", "1")
import numpy as np

B = 65536
E = 4
F = 32
H = 256
NODES = 4
D_MID = 128
D_OUT = 8
N_CORES = 8
CH = 512

_COMPILED = {}
_PLAN_CACHE = {}


def _plan(input_val):
    key = id(input_val)
    oh = np.asarray(input_val)[:, NODES * F:].reshape(B, NODES, E)
    e = np.argmax(oh, axis=2).astype(np.int32)          # [B, 4]
    e0, e1, e2, e3 = e[:, 0], e[:, 1], e[:, 2], e[:, 3]
    ci = ((e1 * 4 + e2) * 4 + e3) * 4 + e0              # combo id, (e1,e2,e3,e0) lex
    cnt = np.bincount(ci, minlength=256)
    npc = (cnt + N_CORES - 1) // N_CORES                # per-core cols per combo
    off = np.zeros(257, np.int64)
    off[1:] = np.cumsum(npc)
    W = int(off[256])

    order = np.argsort(ci, kind="stable")               # rows grouped by combo
    col_of_row = np.empty(B, np.int64)
    core_of_row = np.empty(B, np.int64)
    pos = 0
    for c in range(256):
        n = int(cnt[c])
        rows = order[pos:pos + n]
        pos += n
        k = np.arange(n)
        core_of_row[rows] = k % N_CORES
        col_of_row[rows] = off[c] + k // N_CORES

    def ej(c, j):
        return (c & 3) if j == 0 else (c >> 6) if j == 1 \
            else ((c >> 4) & 3) if j == 2 else ((c >> 2) & 3)

    runs = []
    for j in range(4):
        rj = []
        for c in range(256):
            a, b = int(off[c]), int(off[c + 1])
            if a == b:
                continue
            x = ej(c, j)
            if rj and rj[-1][0] == x and rj[-1][2] == a:
                rj[-1] = (x, rj[-1][1], b)
            else:
                rj.append((x, a, b))
        runs.append(tuple(tuple(r) for r in rj))

    e3_of_col = np.zeros(W, np.int32)
    for c in range(256):
        e3_of_col[off[c]:off[c + 1]] = (c >> 2) & 3

    return dict(W=W, runs=tuple(runs), col_of_row=col_of_row,
                core_of_row=core_of_row, e3_of_col=e3_of_col)


def _pieces(runs_j, a0, b0):
    out = []
    for (x, a, b) in runs_j:
        if b <= a0 or a >= b0:
            continue
        out.append((x, max(a, a0), min(b, b0)))
    return out


def _build(W, runs, bpre, bmid, bup):
    import concourse.bass as bass  # noqa: F401
    from concourse import bacc
    import concourse.mybir as mybir
    from concourse.tile import TileContext

    F32 = mybir.dt.float32
    F16 = mybir.dt.float16
    ADD = mybir.AluOpType.add
    MAX = mybir.AluOpType.max

    nc = bacc.Bacc("TRN2", target_bir_lowering=False, debug=False,
                   num_devices=N_CORES)
    xfall = nc.dram_tensor("xfall", [128, W], F16, kind="ExternalInput").ap()
    winall = nc.dram_tensor("winall", [32, 4096], F16, kind="ExternalInput").ap()
    waall = nc.dram_tensor("waall", [128, 3072], F16, kind="ExternalInput").ap()
    wball = nc.dram_tensor("wball", [128, 6144], F16, kind="ExternalInput").ap()
    wuall = nc.dram_tensor("wuall", [128, 3072], F16, kind="ExternalInput").ap()
    w3all = nc.dram_tensor("w3all", [128, 64], F16, kind="ExternalInput").ap()
    bvals = nc.dram_tensor("bvals", [128, 10], F32, kind="ExternalInput").ap()
    yT = nc.dram_tensor("yT", [8, W], F32, kind="ExternalOutput").ap()

    chunks = [(a, min(a + CH, W)) for a in range(0, W, CH)]
    nch = len(chunks)

    with TileContext(nc) as tc:
        with (
            tc.tile_pool(name="wpool", bufs=1) as wp,
            tc.tile_pool(name="big", bufs=1) as bigp,
            tc.tile_pool(name="xfp", bufs=4) as xfp,
            tc.tile_pool(name="acts", bufs=1) as ap_,
            tc.tile_pool(name="ppre", bufs=2, space="PSUM") as ppre,
            tc.tile_pool(name="pmid", bufs=1, space="PSUM") as pmid,
            tc.tile_pool(name="pup", bufs=1, space="PSUM") as pup,
            tc.tile_pool(name="phd", bufs=1, space="PSUM") as phd,
        ):
            def wtile(shape, tag, src):
                t = wp.tile(shape, F16, tag=tag, name=tag)
                nc.gpsimd.dma_start(t[:, :], src)
                return t

            winall_t = wtile([32, 4096], "winall", winall[:, :])

            state = {}

            def fetch_xf(j, ci):
                if ci >= nch or (j, ci) in state:
                    return
                a0, b0 = chunks[ci]
                t = xfp.tile([32, CH], F16, tag=f"xf{j}", name=f"xf{j}")
                nc.sync.dma_start(t[:, 0:b0 - a0], xfall[32 * j:32 * (j + 1), a0:b0])
                state[(j, ci)] = t

            for _c in (0, 1):
                for _j in range(4):
                    fetch_xf(_j, _c)
            wu_all = wtile([128, 3072], "wuall", wuall[:, :])
            wb_all = wtile([128, 6144], "wball", wball[:, :])
            wa_all = wtile([128, 3072], "waall", waall[:, :])
            w3_all = wtile([128, 64], "w3all", w3all[:, :])

            def win_t(i):
                return winall_t[:, i * 256:(i + 1) * 256]

            def wb_t(i, k):
                return wb_all[:, (2 * i + k) * 256:(2 * i + k + 1) * 256]

            def wa_t(i):
                return wa_all[:, i * 256:(i + 1) * 256]

            def wu_t(i, k):
                return wu_all[:, (2 * i + k) * 128:(2 * i + k + 1) * 128]

            def w3u_t(e, k):
                return w3_all[:, (2 * e + k) * 8:(2 * e + k + 1) * 8]

            xp_t = [bigp.tile([128, W], F16, tag=f"xp{j}", name=f"xp{j}")
                    for j in range(3)]
            bv_t = bigp.tile([128, 10], F32, tag="bvals", name="bv_t")
            nc.gpsimd.dma_start(bv_t[:, :], bvals[:, :])
            BIDX = {}
            for _i, _k in enumerate(("p0", "p1", "p2", "p3", "m1", "m2", "m3",
                                     "u0", "u1", "u2")):
                BIDX[_k] = _i

            HB = {0: 3, 1: 4, 2: 6, 3: 8}      # hin buf depth per node
            RELU = mybir.ActivationFunctionType.Relu
            eng_busy = {"vector": 0.0, "scalar": 0.0}
            EW = {"vector": (1.04, 170.0), "scalar": (0.833, 175.0)}

            def emit_relu(out_ap, in_ap, bias, nfree, bkey):
                costs = {e: nfree * w + f for e, (w, f) in EW.items()}
                e = min(eng_busy, key=lambda k: eng_busy[k] + costs[k])
                eng_busy[e] += costs[e]
                if e == "scalar":
                    bi = BIDX[bkey]
                    nc.scalar.activation(out_ap, in_ap, RELU,
                                         bias=bv_t[0:in_ap.partition_size(),
                                                   bi:bi + 1])
                else:
                    getattr(nc, e).tensor_scalar(out_ap, in_ap, float(bias),
                                                 0.0, ADD, MAX)

            def emit_pre(j, ci):
                a0, b0 = chunks[ci]
                L = b0 - a0
                if ci == 0:
                    fetch_xf(j, 0)
                    fetch_xf(j, 1)
                fetch_xf(j, ci + 2)
                xt = state.pop((j, ci))
                ps = ppre.tile([128, 2, CH], F32, tag="pre", name="pre")
                for (ex, pa, pb) in _pieces(runs[j], a0, b0):
                    ra, rb = pa - a0, pb - a0
                    for m in (0, 1):
                        nc.tensor.matmul(
                            ps[:, m, ra:rb],
                            win_t(4 * j + ex)[:, m * 128:(m + 1) * 128],
                            xt[:, ra:rb], start=True, stop=True)
                h = ap_.tile([128, 2, CH], F16, tag=f"h{j}", name=f"h{j}",
                             bufs=HB[j])
                emit_relu(h[:, :, 0:L], ps[:, :, 0:L], bpre[j], 2 * L, f"p{j}")
                state[("h", j, ci)] = h

            def emit_mid(j, ci):
                a0, b0 = chunks[ci]
                L = b0 - a0
                hp = state.pop(("h", j, ci))
                ps = pmid.tile([128, 2, CH], F32, tag="mid", name="mid")
                for (ex, pa, pb) in _pieces(runs[j], a0, b0):
                    ra, rb = pa - a0, pb - a0
                    i12 = 4 * (j - 1) + ex
                    for m in (0, 1):
                        sl = slice(m * 128, (m + 1) * 128)
                        nc.tensor.matmul(ps[:, m, ra:rb], wb_t(i12, 0)[:, sl],
                                         hp[:, 0, ra:rb], start=True, stop=False)
                        nc.tensor.matmul(ps[:, m, ra:rb], wb_t(i12, 1)[:, sl],
                                         hp[:, 1, ra:rb], start=False, stop=False)
                        nc.tensor.matmul(ps[:, m, ra:rb], wa_t(i12)[:, sl],
                                         xp_t[j - 1][:, pa:pb],
                                         start=False, stop=True)
                h = ap_.tile([128, 2, CH], F16, tag=f"hm{j}", name=f"hm{j}",
                             bufs=3)
                emit_relu(h[:, :, 0:L], ps[:, :, 0:L], bmid[j], 2 * L, f"m{j}")
                state[("hm", j, ci)] = h

            def emit_up(j, ci):
                a0, b0 = chunks[ci]
                L = b0 - a0
                hp = state.pop(("hm", j, ci) if j > 0 else ("h", j, ci))
                ps = pup.tile([128, CH], F32, tag="up", name="up")
                for (ex, pa, pb) in _pieces(runs[j], a0, b0):
                    ra, rb = pa - a0, pb - a0
                    nc.tensor.matmul(ps[:, ra:rb], wu_t(4 * j + ex, 0),
                                     hp[:, 0, ra:rb], start=True, stop=False)
                    nc.tensor.matmul(ps[:, ra:rb], wu_t(4 * j + ex, 1),
                                     hp[:, 1, ra:rb], start=False, stop=True)
                emit_relu(xp_t[j][:, a0:b0], ps[:, 0:L], bup[j], L, f"u{j}")

            def emit_head(ci):
                a0, b0 = chunks[ci]
                L = b0 - a0
                hp = state.pop(("hm", 3, ci))
                ps = phd.tile([8, CH], F32, tag="hd", name="hd")
                for (ex, pa, pb) in _pieces(runs[3], a0, b0):
                    ra, rb = pa - a0, pb - a0
                    nc.tensor.matmul(ps[:, ra:rb], w3u_t(ex, 0),
                                     hp[:, 0, ra:rb], start=True, stop=False)
                    nc.tensor.matmul(ps[:, ra:rb], w3u_t(ex, 1),
                                     hp[:, 1, ra:rb], start=False, stop=True)
                yc = ap_.tile([8, CH], F32, tag="yc", name="yc", bufs=2)
                nc.any.tensor_scalar(yc[:, 0:L], ps[:, 0:L], 0.0, None, ADD)
                nc.sync.dma_start(yT[:, a0:b0], yc[:, 0:L])

            STAGES = [
                (lambda c: emit_pre(0, c), 0),
                (lambda c: emit_up(0, c), 1),
                (lambda c: emit_pre(1, c), 0),
                (lambda c: emit_mid(1, c), 2),
                (lambda c: emit_pre(2, c), 0),
                (lambda c: emit_up(1, c), 3),
                (lambda c: emit_pre(3, c), 0),
                (lambda c: emit_mid(2, c), 4),
                (lambda c: emit_up(2, c), 5),
                (lambda c: emit_mid(3, c), 6),
                (lambda c: emit_head(c), 7),
            ]
            for step in range(nch + 8):
                for fn, skew in STAGES:
                    c = step - skew
                    if 0 <= c < nch:
                        fn(c)
    nc.compile()
    return nc


def _bias_scalar(b):
    b = np.asarray(b, np.float32)
    assert np.ptp(b) == 0.0, "bias must be a single constant"
    return float(b.flat[0])


def _prep_inputs(p):
    f16 = np.float16
    plan = _plan(p["input_val"])
    W = plan["W"]

    winall = np.empty((32, 4096), f16)
    in_w = [p["W0_0"], p["W1_pre"], p["W2_pre"], p["W3_pre"]]
    for j in range(4):
        for e in range(4):
            i = 4 * j + e
            winall[:, i * 256:(i + 1) * 256] = in_w[j][e]

    waall = np.empty((128, 3072), f16)
    wball = np.empty((128, 6144), f16)
    for j in (1, 2, 3):
        w0 = p[f"W{j}_0"]
        for e in range(4):
            i = 4 * (j - 1) + e
            waall[:, i * 256:(i + 1) * 256] = w0[e][0:128, :]
            for k in range(2):
                wball[:, (2 * i + k) * 256:(2 * i + k + 1) * 256] = \
                    w0[e][128 + 128 * k:256 + 128 * k, :]

    wuall = np.empty((128, 3072), f16)
    for j in (0, 1, 2):
        w1 = p[f"W{j}_1"]
        for e in range(4):
            i = 4 * j + e
            for k in range(2):
                wuall[:, (2 * i + k) * 128:(2 * i + k + 1) * 128] = \
                    w1[e][128 * k:128 * (k + 1), :]

    w3all = np.zeros((128, 64), f16)
    w3 = np.asarray(p["W3_1"]).astype(f16)
    for e in range(4):
        for k in range(2):
            w3all[:, (2 * e + k) * 8:(2 * e + k + 1) * 8] = \
                w3[e][128 * k:128 * (k + 1), :]

    xs = np.asarray(p["input_val"])[:, 0:NODES * F].astype(f16)     # [B, 128]

    bv = np.zeros((128, 10), np.float32)
    for i, k in enumerate(("b0_0", "b1_pre", "b2_pre", "b3_pre",
                           "b1_0", "b2_0", "b3_0",
                           "b0_1", "b1_1", "b2_1")):
        bv[:, i] = np.asarray(p[k], np.float32).flat[0]
    shared = dict(winall=winall, waall=waall, wball=wball, wuall=wuall,
                  w3all=w3all, bvals=bv)
    in_maps = []
    for c in range(N_CORES):
        rows = np.where(plan["core_of_row"] == c)[0]
        cols = plan["col_of_row"][rows]
        xf = np.zeros((128, W), f16)
        xf[:, cols] = xs[rows].T
        m = dict(shared)
        m["xfall"] = xf
        in_maps.append(m)
    return in_maps


def kernel(**inputs):
    from concourse.bass_utils import run_bass_kernel_spmd

    p = {k: np.asarray(v) for k, v in inputs.items()}
    plan = _plan(p["input_val"])
    bpre = [_bias_scalar(p[k]) for k in ("b0_0", "b1_pre", "b2_pre", "b3_pre")]
    bmid = {j: _bias_scalar(p[f"b{j}_0"]) for j in (1, 2, 3)}
    bup = {j: _bias_scalar(p[f"b{j}_1"]) for j in (0, 1, 2)}

    key = (plan["W"], plan["runs"], tuple(bpre),
           tuple(sorted(bmid.items())), tuple(sorted(bup.items())))
    if key not in _COMPILED:
        _COMPILED[key] = _build(plan["W"], plan["runs"], bpre, bmid, bup)
        _COMPILED[("nc", True)] = _COMPILED[key]
    nc = _COMPILED[key]

    in_maps = _prep_inputs(p)
    res = run_bass_kernel_spmd(nc, in_maps, core_ids=list(range(N_CORES)))

    out = np.empty((B, D_OUT), np.float32)
    for c in range(N_CORES):
        y = res.results[c]["yT"]                                    # [8, W]
        rows = np.where(plan["core_of_row"] == c)[0]
        cols = plan["col_of_row"][rows]
        out[rows] = y[:, cols].T
    oh = p["input_val"][:, NODES * F:].reshape(B, NODES, E)
    e3 = np.argmax(oh[:, 3], axis=1)
    out += np.asarray(p["b3_1"], np.float32)[e3]
    return out
